# revision 4
# baseline (speedup 1.0000x reference)
"""Newton-SOR batched solver for Trainium2, 8 NeuronCores, data parallel.

Math: the reference's while-loop always runs all MAXITER=16 iterations
(the fp32 residual-norm floor ~5e-5 never reaches TOL=1e-6), and the
iterate converges to the fixed point F(x*)=0, so an inner solve that is
approximate but convergent reproduces the reference to ~1e-5 relative.

Per outer iteration (damped Newton-Jacobi form, K=1 Neumann):
    d~ = diag(A) + 3 x^2
    v  = omega * F / d~          (rounded to bf16; update uses the SAME
                                  rounded vector for exact consistency)
    x' = x - v
    F' = F - A @ v + (x'^3 - x^3)

The only heavy op is the per-element matvec A @ v (2048 independent
128x128 matrices). These run on TensorE as bf16 self-loading matmuls
(N=1) with fp32 PSUM accumulation (~80ns/element): A = A1 + A2 with
both parts bf16; F is carried incrementally with A1-only applies, and
the *exactly linear* accumulated drift A2 @ (x - x_anchor) is folded in
at a few correction iterations. Everything stays in "transposed" layout
[var, element] so TensorE needs no transposes at all; VectorE does the
pointwise chain per half-tile so it pipelines under TensorE's stream.
"""

import numpy as np
import ml_dtypes

BATCH = 2048
N = 128
NCORES = 8
PER_CORE = BATCH // NCORES          # 256
NTILES = 2                          # halves of 128 elements each
TPE = PER_CORE // NTILES            # 128 elements per tile
NITER = 16
CORR_ITERS = frozenset({6, 10, 13, 15})

_BF16 = ml_dtypes.bfloat16

_compiled = None


def _build():
    import concourse.bacc as bacc
    import concourse.mybir as mybir
    from concourse.tile import TileContext

    f32 = mybir.dt.float32
    bf16 = mybir.dt.bfloat16
    op = mybir.AluOpType

    nc = bacc.Bacc("TRN2", target_bir_lowering=False, debug=False)

    # DRAM I/O (per-core shard, transposed layouts)
    at1 = [
        nc.dram_tensor(f"at1_{t}", [N, TPE * N], bf16, kind="ExternalInput")
        for t in range(NTILES)
    ]
    at2 = [
        nc.dram_tensor(f"at2_{t}", [N, TPE * N], bf16, kind="ExternalInput")
        for t in range(NTILES)
    ]
    x0_d = nc.dram_tensor("x0t", [N, PER_CORE], f32, kind="ExternalInput")
    b_d = nc.dram_tensor("bt", [N, PER_CORE], f32, kind="ExternalInput")
    da_d = nc.dram_tensor("dat", [N, PER_CORE], f32, kind="ExternalInput")
    om_d = nc.dram_tensor("omt", [N, PER_CORE], f32, kind="ExternalInput")
    out_d = nc.dram_tensor("outt", [N, PER_CORE], f32, kind="ExternalOutput")

    with TileContext(nc) as tc:
        with (
            tc.tile_pool(name="wts", bufs=1) as wts,
            tc.tile_pool(name="vec", bufs=1) as vec,
            tc.tile_pool(name="roll", bufs=2) as roll,
            tc.tile_pool(name="ps", bufs=2, space="PSUM") as psp,
        ):
            a1_sb = []
            a2_sb = []
            for t in range(NTILES):
                a1_t = wts.tile([N, TPE * N], bf16, name=f"a1sb{t}", tag=f"a1{t}")
                nc.gpsimd.dma_start(a1_t[:, :], at1[t][:, :])
                a1_sb.append(a1_t)
            for t in range(NTILES):
                a2_t = wts.tile([N, TPE * N], bf16, name=f"a2sb{t}", tag=f"a2{t}")
                nc.gpsimd.dma_start(a2_t[:, :], at2[t][:, :])
                a2_sb.append(a2_t)

            # Long-lived per-tile state, all [N, TPE] fp32 in SBUF.
            x_t, F_t, b_t, da_t, om_t, x2_t, x3_t, wacc_t = (
                [None] * NTILES for _ in range(8)
            )
            for t in range(NTILES):
                cs = slice(t * TPE, (t + 1) * TPE)
                x_t[t] = vec.tile([N, TPE], f32, name=f"x{t}", tag=f"x{t}")
                nc.gpsimd.dma_start(x_t[t][:, :], x0_d[:, cs])
                b_t[t] = vec.tile([N, TPE], f32, name=f"b{t}", tag=f"b{t}")
                nc.gpsimd.dma_start(b_t[t][:, :], b_d[:, cs])
                da_t[t] = vec.tile([N, TPE], f32, name=f"da{t}", tag=f"da{t}")
                nc.gpsimd.dma_start(da_t[t][:, :], da_d[:, cs])
                om_t[t] = vec.tile([N, TPE], f32, name=f"om{t}", tag=f"om{t}")
                nc.gpsimd.dma_start(om_t[t][:, :], om_d[:, cs])
                F_t[t] = vec.tile([N, TPE], f32, name=f"F{t}", tag=f"F{t}")
                x2_t[t] = vec.tile([N, TPE], f32, name=f"x2{t}", tag=f"x2{t}")
                x3_t[t] = vec.tile([N, TPE], f32, name=f"x3{t}", tag=f"x3{t}")
                wacc_t[t] = vec.tile([N, TPE], f32, name=f"wa{t}", tag=f"wa{t}")
                nc.vector.memset(wacc_t[t][:, :], 0.0)

            def apply_mms(ps, a_sb, v_bf, start, stop):
                for e in range(TPE):
                    nc.tensor.matmul(
                        ps[:, e : e + 1],
                        a_sb[:, e * N : (e + 1) * N],
                        v_bf[:, e : e + 1],
                        start=start,
                        stop=stop,
                    )

            # ---- init: x <- bf16(x0); F = A@x + x^3 - b ----
            # NOTE: start=True clears has_written for the WHOLE psum bank,
            # so A1/A2 contributions go to separate psum tiles (banks) and
            # are merged on VectorE.
            ps_t = [None] * NTILES
            ps2_t = [None] * NTILES
            for t in range(NTILES):
                xb = roll.tile([N, TPE], bf16, name=f"xb{t}", tag=f"vb{t}")
                nc.vector.tensor_copy(xb[:, :], x_t[t][:, :])
                # make carried x exactly the rounded value
                nc.vector.tensor_copy(x_t[t][:, :], xb[:, :])
                ps = psp.tile([N, TPE], f32, name=f"psi{t}", tag=f"ps{t}")
                apply_mms(ps, a1_sb[t], xb, True, True)
                ps2 = psp.tile([N, TPE], f32, name=f"psj{t}", tag=f"pc{t}")
                apply_mms(ps2, a2_sb[t], xb, True, True)
                ps_t[t] = ps
                ps2_t[t] = ps2
            for t in range(NTILES):
                nc.vector.tensor_mul(x2_t[t][:, :], x_t[t][:, :], x_t[t][:, :])
                nc.vector.tensor_mul(x3_t[t][:, :], x2_t[t][:, :], x_t[t][:, :])
                # F = (x3 - b) + ps + ps2
                nc.vector.tensor_sub(F_t[t][:, :], x3_t[t][:, :], b_t[t][:, :])
                nc.vector.tensor_add(F_t[t][:, :], F_t[t][:, :], ps_t[t][:, :])
                nc.vector.tensor_add(F_t[t][:, :], F_t[t][:, :], ps2_t[t][:, :])

            # ---- 16 outer iterations ----
            for k in range(1, NITER + 1):
                corr = k in CORR_ITERS
                for t in range(NTILES):
                    # d~ = 3*x2 + dA ; r = 1/d~ ; v = F * (om * r)
                    dt_ = roll.tile([N, TPE], f32, name=f"dt{t}", tag=f"dt{t}")
                    nc.vector.scalar_tensor_tensor(
                        dt_[:, :], x2_t[t][:, :], 3.0, da_t[t][:, :],
                        op0=op.mult, op1=op.add,
                    )
                    nc.vector.reciprocal(dt_[:, :], dt_[:, :])
                    nc.vector.tensor_mul(dt_[:, :], dt_[:, :], om_t[t][:, :])
                    v = roll.tile([N, TPE], f32, name=f"v{t}", tag=f"v{t}")
                    nc.vector.tensor_mul(v[:, :], F_t[t][:, :], dt_[:, :])
                    v_bf = roll.tile([N, TPE], bf16, name=f"vb{t}", tag=f"vb{t}")
                    nc.vector.tensor_copy(v_bf[:, :], v[:, :])
                    # rounded v back to f32 for the exact-consistency updates
                    nc.vector.tensor_copy(v[:, :], v_bf[:, :])
                    # x' = x - v
                    nc.vector.tensor_sub(x_t[t][:, :], x_t[t][:, :], v[:, :])

                    ps = psp.tile([N, TPE], f32, name=f"psk{t}", tag=f"ps{t}")
                    ps2 = None
                    if corr:
                        # w = round(wacc + v); wacc' = (wacc + v) - w
                        w32 = roll.tile([N, TPE], f32, name=f"w{t}", tag=f"w{t}")
                        nc.vector.tensor_add(
                            w32[:, :], wacc_t[t][:, :], v[:, :]
                        )
                        w_bf = roll.tile(
                            [N, TPE], bf16, name=f"wb{t}", tag=f"wb{t}"
                        )
                        nc.vector.tensor_copy(w_bf[:, :], w32[:, :])
                        apply_mms(ps, a1_sb[t], v_bf, True, True)
                        ps2 = psp.tile([N, TPE], f32, name=f"psc{t}", tag=f"pc{t}")
                        apply_mms(ps2, a2_sb[t], w_bf, True, True)
                        wr = roll.tile([N, TPE], f32, name=f"wr{t}", tag=f"v{t}")
                        nc.vector.tensor_copy(wr[:, :], w_bf[:, :])
                        nc.vector.tensor_sub(
                            wacc_t[t][:, :], w32[:, :], wr[:, :]
                        )
                    else:
                        nc.vector.tensor_add(
                            wacc_t[t][:, :], wacc_t[t][:, :], v[:, :]
                        )
                        apply_mms(ps, a1_sb[t], v_bf, True, True)

                    # cubes and F update
                    nc.vector.tensor_mul(
                        x2_t[t][:, :], x_t[t][:, :], x_t[t][:, :]
                    )
                    nx3 = roll.tile([N, TPE], f32, name=f"nx3{t}", tag=f"dt{t}")
                    nc.vector.tensor_mul(
                        nx3[:, :], x2_t[t][:, :], x_t[t][:, :]
                    )
                    # F' = F + (nx3 - x3) - ps [- ps2]
                    nc.vector.tensor_sub(x3_t[t][:, :], nx3[:, :], x3_t[t][:, :])
                    nc.vector.tensor_add(F_t[t][:, :], F_t[t][:, :], x3_t[t][:, :])
                    nc.vector.tensor_sub(F_t[t][:, :], F_t[t][:, :], ps[:, :])
                    if ps2 is not None:
                        nc.vector.tensor_sub(F_t[t][:, :], F_t[t][:, :], ps2[:, :])
                    # x3 <- nx3
                    nc.vector.tensor_copy(x3_t[t][:, :], nx3[:, :])

            for t in range(NTILES):
                cs = slice(t * TPE, (t + 1) * TPE)
                nc.gpsimd.dma_start(out_d[:, cs], x_t[t][:, :])

    nc.compile()
    return nc


def _get_compiled():
    global _compiled
    if _compiled is None:
        _compiled = _build()
    return _compiled


def _prep_inputs(x, A, b, omega):
    """Host-side shard + layout prep. Returns list of per-core in_maps."""
    A = np.ascontiguousarray(A, dtype=np.float32)
    x = np.asarray(x, dtype=np.float32)
    b = np.asarray(b, dtype=np.float32)
    omega = np.asarray(omega, dtype=np.float32)

    A1 = A.astype(_BF16)
    A2 = (A - A1.astype(np.float32)).astype(_BF16)
    dA = np.ascontiguousarray(np.diagonal(A, axis1=1, axis2=2))

    in_maps = []
    for c in range(NCORES):
        sl = slice(c * PER_CORE, (c + 1) * PER_CORE)
        m = {}
        for t in range(NTILES):
            ts = slice(c * PER_CORE + t * TPE, c * PER_CORE + (t + 1) * TPE)
            # lhsT layout [j, (e, i)]: element e's weights = A[e].T
            m[f"at1_{t}"] = np.ascontiguousarray(
                A1[ts].transpose(2, 0, 1)
            ).reshape(N, TPE * N)
            m[f"at2_{t}"] = np.ascontiguousarray(
                A2[ts].transpose(2, 0, 1)
            ).reshape(N, TPE * N)
        m["x0t"] = np.ascontiguousarray(x[sl].T)
        m["bt"] = np.ascontiguousarray(b[sl].T)
        m["dat"] = np.ascontiguousarray(dA[sl].T)
        m["omt"] = np.ascontiguousarray(
            np.broadcast_to(omega[sl].reshape(1, PER_CORE), (N, PER_CORE))
        )
        in_maps.append(m)
    return in_maps


def _run(inputs, trace=False):
    from concourse.bass_utils import run_bass_kernel_spmd

    nc = _get_compiled()
    in_maps = _prep_inputs(
        inputs["x"], inputs["A"], inputs["b"], inputs["omega"]
    )
    res = run_bass_kernel_spmd(
        nc, in_maps, core_ids=list(range(NCORES)), trace=trace
    )
    out = np.empty((BATCH, N), dtype=np.float32)
    for c in range(NCORES):
        out[c * PER_CORE : (c + 1) * PER_CORE] = res.results[c]["outt"].T
    return out, res


def kernel(x, A, b, omega):
    out, _ = _run({"x": x, "A": A, "b": b, "omega": omega}, trace=False)
    return out


# revision 5
# speedup vs baseline: 1.3423x; 1.3423x over previous
"""Newton-SOR batched solver for Trainium2, 8 NeuronCores, data parallel.

Math: the reference's while-loop always runs all MAXITER=16 iterations
(the fp32 residual-norm floor ~5e-5 never reaches TOL=1e-6), and the
iterate converges to the fixed point F(x*)=0, so an approximate-but-
convergent inner solve reproduces the reference to ~1e-5 relative.

Per outer iteration (damped Newton-Jacobi, K=1 Neumann):
    d~ = diag(A) + 3 x^2
    v  = omega * F / d~      (rounded to bf16; the update uses the SAME
                              rounded vector, keeping F exactly consistent)
    x' = x - v
    F' = F - A @ v + (x'^3 - x^3)

The heavy op is 2048 independent 128x128 matvecs per iteration. They run
on TensorE as bf16 self-loading matmuls (N=1) with fp32 PSUM accumulation
(~32ns/element steady): A = A1 + A2, both bf16; F is carried with
A1-only applies and the *exactly linear* deferred part A2 @ (sum(v)-x0)
is folded in at a few correction iterations (drift contracts afterwards).
Everything stays in transposed layout [var, element] so TensorE needs no
transposes; VectorE/ScalarE pointwise work is hoisted off the PSUM
critical path so it hides under TensorE's stream. The 16th iteration
needs no matvec at all (F_16 is never consumed).
"""

import numpy as np
import ml_dtypes

BATCH = 2048
N = 128
NCORES = 8
PER_CORE = BATCH // NCORES          # 256
NTILES = 2                          # halves of 128 elements each
TPE = PER_CORE // NTILES            # 128 elements per tile
NITER = 16
CORR_ITERS = frozenset({6, 11, 15})
RECIP_FULL = 2                      # full reciprocal for k <= this
RECIP_NEWTON = 5                    # one Newton refresh for k <= this
NCHUNK = 4                          # DMA chunks per A1 tile

_BF16 = ml_dtypes.bfloat16

_compiled = None


def _build():
    import concourse.bacc as bacc
    import concourse.mybir as mybir
    from concourse.tile import TileContext

    f32 = mybir.dt.float32
    bf16 = mybir.dt.bfloat16
    op = mybir.AluOpType

    nc = bacc.Bacc("TRN2", target_bir_lowering=False, debug=False)

    at1 = [
        nc.dram_tensor(f"at1_{t}", [N, TPE * N], bf16, kind="ExternalInput")
        for t in range(NTILES)
    ]
    at2 = [
        nc.dram_tensor(f"at2_{t}", [N, TPE * N], bf16, kind="ExternalInput")
        for t in range(NTILES)
    ]
    x0_d = nc.dram_tensor("x0t", [N, PER_CORE], f32, kind="ExternalInput")
    b_d = nc.dram_tensor("bt", [N, PER_CORE], f32, kind="ExternalInput")
    da_d = nc.dram_tensor("dat", [N, PER_CORE], f32, kind="ExternalInput")
    om_d = nc.dram_tensor("omt", [N, PER_CORE], f32, kind="ExternalInput")
    out_d = nc.dram_tensor("outt", [N, PER_CORE], f32, kind="ExternalOutput")

    with TileContext(nc) as tc:
        with (
            tc.tile_pool(name="wts", bufs=1) as wts,
            tc.tile_pool(name="vec", bufs=1) as vec,
            tc.tile_pool(name="roll", bufs=2) as roll,
            tc.tile_pool(name="ps", bufs=2, space="PSUM") as psp,
        ):
            # small vectors first so pointwise prep can start immediately
            x0_sb = vec.tile([N, PER_CORE], f32, name="x0sb")
            nc.sync.dma_start(x0_sb[:, :], x0_d[:, :])
            b_sb = vec.tile([N, PER_CORE], f32, name="bsb")
            nc.sync.dma_start(b_sb[:, :], b_d[:, :])
            da_sb = vec.tile([N, PER_CORE], f32, name="dasb")
            nc.sync.dma_start(da_sb[:, :], da_d[:, :])
            om_sb = vec.tile([N, PER_CORE], f32, name="omsb")
            nc.sync.dma_start(om_sb[:, :], om_d[:, :])

            # A1 weights, chunked, split across two DMA engines
            a1_sb = []
            for t in range(NTILES):
                a1_t = wts.tile([N, TPE * N], bf16, name=f"a1sb{t}", tag=f"a1{t}")
                a1_sb.append(a1_t)
            csz = TPE * N // NCHUNK
            for q in range(NCHUNK):
                for t in range(NTILES):
                    eng = nc.sync if t == 0 else nc.gpsimd
                    eng.dma_start(
                        a1_sb[t][:, q * csz : (q + 1) * csz],
                        at1[t][:, q * csz : (q + 1) * csz],
                    )
            a2_sb = []
            for t in range(NTILES):
                a2_t = wts.tile([N, TPE * N], bf16, name=f"a2sb{t}", tag=f"a2{t}")
                eng = nc.sync if t == 0 else nc.gpsimd
                eng.dma_start(a2_t[:, :], at2[t][:, :])
                a2_sb.append(a2_t)

            def apply_mms(ps, a_sb, v_bf):
                for e in range(TPE):
                    nc.tensor.matmul(
                        ps[:, e : e + 1],
                        a_sb[:, e * N : (e + 1) * N],
                        v_bf[:, e : e + 1],
                        start=True,
                        stop=True,
                    )

            # per-tile persistent state
            F_t = [vec.tile([N, TPE], f32, name=f"F{t}") for t in range(2)]
            wa_t = [vec.tile([N, TPE], f32, name=f"wa{t}") for t in range(2)]
            r_t = [vec.tile([N, TPE], f32, name=f"r{t}") for t in range(2)]
            s_t = [vec.tile([N, TPE], f32, name=f"s{t}") for t in range(2)]
            x_t = [None] * NTILES
            x3_t = [None] * NTILES
            v_t = [None] * NTILES
            vb_t = [None] * NTILES

            # ---- init per tile ----
            for t in range(NTILES):
                cs = slice(t * TPE, (t + 1) * TPE)
                xb = roll.tile([N, TPE], bf16, name=f"xb{t}", tag=f"vb{t}")
                nc.scalar.copy(xb[:, :], x0_sb[:, cs])
                x = roll.tile([N, TPE], f32, name=f"x{t}", tag=f"x{t}")
                nc.scalar.copy(x[:, :], xb[:, :])          # x = round(x0)
                ps = psp.tile([N, TPE], f32, name=f"psi{t}", tag=f"ps{t}")
                apply_mms(ps, a1_sb[t], xb)
                # wacc = -x
                nc.vector.tensor_scalar_mul(wa_t[t][:, :], x[:, :], -1.0)
                x2 = roll.tile([N, TPE], f32, name=f"x2{t}", tag=f"x2{t}")
                nc.scalar.square(x2[:, :], x[:, :])
                x3 = roll.tile([N, TPE], f32, name=f"x3{t}", tag=f"x3{t}")
                nc.vector.tensor_mul(x3[:, :], x2[:, :], x[:, :])
                # d~, r, s
                dt_ = roll.tile([N, TPE], f32, name=f"dt{t}", tag=f"dt{t}")
                nc.vector.scalar_tensor_tensor(
                    dt_[:, :], x2[:, :], 3.0, da_sb[:, cs],
                    op0=op.mult, op1=op.add,
                )
                nc.vector.reciprocal(r_t[t][:, :], dt_[:, :])
                nc.vector.tensor_mul(s_t[t][:, :], r_t[t][:, :], om_sb[:, cs])
                # F = (x3 - b) + ps
                nc.vector.tensor_sub(F_t[t][:, :], x3[:, :], b_sb[:, cs])
                nc.vector.tensor_add(F_t[t][:, :], F_t[t][:, :], ps[:, :])
                # v_1
                v = roll.tile([N, TPE], f32, name=f"v{t}", tag=f"v{t}")
                nc.vector.tensor_mul(v[:, :], F_t[t][:, :], s_t[t][:, :])
                v_bf = roll.tile([N, TPE], bf16, name=f"vb{t}", tag=f"vb{t}")
                nc.scalar.copy(v_bf[:, :], v[:, :])
                nc.scalar.copy(v[:, :], v_bf[:, :])        # rounded, f32
                x_t[t], x3_t[t], v_t[t], vb_t[t] = x, x3, v, v_bf

            # ---- iterations 1..15 (16th needs no apply) ----
            for k in range(1, NITER):
                corr = k in CORR_ITERS
                for t in range(NTILES):
                    cs = slice(t * TPE, (t + 1) * TPE)
                    x, x3, v, v_bf = x_t[t], x3_t[t], v_t[t], vb_t[t]
                    F, wa, r, s = F_t[t], wa_t[t], r_t[t], s_t[t]

                    ps = psp.tile([N, TPE], f32, name=f"psk{t}_{k}", tag=f"ps{t}")
                    apply_mms(ps, a1_sb[t], v_bf)

                    # --- hoisted pointwise (runs under the PE stream) ---
                    xn = roll.tile([N, TPE], f32, name=f"x{t}_{k}", tag=f"x{t}")
                    nc.vector.tensor_sub(xn[:, :], x[:, :], v[:, :])
                    x2 = roll.tile([N, TPE], f32, name=f"x2{t}_{k}", tag=f"x2{t}")
                    nc.scalar.square(x2[:, :], xn[:, :])
                    nx3 = roll.tile([N, TPE], f32, name=f"x3{t}_{k}", tag=f"x3{t}")
                    nc.vector.tensor_mul(nx3[:, :], x2[:, :], xn[:, :])
                    dc = roll.tile([N, TPE], f32, name=f"dc{t}_{k}", tag=f"dt{t}")
                    nc.vector.tensor_sub(dc[:, :], nx3[:, :], x3[:, :])
                    nc.vector.tensor_add(F[:, :], F[:, :], dc[:, :])
                    ps2 = None
                    if corr:
                        w32 = roll.tile([N, TPE], f32, name=f"w{t}_{k}", tag=f"w{t}")
                        nc.vector.tensor_add(w32[:, :], wa[:, :], v[:, :])
                        w_bf = roll.tile(
                            [N, TPE], bf16, name=f"wb{t}_{k}", tag=f"wb{t}"
                        )
                        nc.scalar.copy(w_bf[:, :], w32[:, :])
                        wr = roll.tile([N, TPE], f32, name=f"wr{t}_{k}", tag=f"w{t}")
                        nc.scalar.copy(wr[:, :], w_bf[:, :])
                        nc.vector.tensor_sub(wa[:, :], w32[:, :], wr[:, :])
                        ps2 = psp.tile(
                            [N, TPE], f32, name=f"psc{t}_{k}", tag=f"pc{t}"
                        )
                        apply_mms(ps2, a2_sb[t], w_bf)
                    else:
                        nc.gpsimd.tensor_add(wa[:, :], wa[:, :], v[:, :])
                    # d~(x'), reciprocal policy, s
                    if k + 1 <= RECIP_NEWTON:
                        dt_ = roll.tile(
                            [N, TPE], f32, name=f"dt{t}_{k}", tag=f"dt{t}"
                        )
                        nc.vector.scalar_tensor_tensor(
                            dt_[:, :], x2[:, :], 3.0, da_sb[:, cs],
                            op0=op.mult, op1=op.add,
                        )
                        if k + 1 <= RECIP_FULL:
                            nc.vector.reciprocal(r[:, :], dt_[:, :])
                        else:
                            # r <- r*(2 - d*r)
                            tmp = roll.tile(
                                [N, TPE], f32, name=f"tm{t}_{k}", tag=f"tm{t}"
                            )
                            nc.vector.tensor_mul(tmp[:, :], dt_[:, :], r[:, :])
                            nc.vector.tensor_scalar(
                                tmp[:, :], tmp[:, :], -1.0, 2.0,
                                op0=op.mult, op1=op.add,
                            )
                            nc.vector.tensor_mul(r[:, :], r[:, :], tmp[:, :])
                        nc.vector.tensor_mul(s[:, :], r[:, :], om_sb[:, cs])

                    # --- PSUM critical path ---
                    nc.vector.tensor_sub(F[:, :], F[:, :], ps[:, :])
                    if ps2 is not None:
                        nc.vector.tensor_sub(F[:, :], F[:, :], ps2[:, :])
                    vn = roll.tile([N, TPE], f32, name=f"v{t}_{k}", tag=f"v{t}")
                    nc.vector.tensor_mul(vn[:, :], F[:, :], s[:, :])
                    vbn = roll.tile([N, TPE], bf16, name=f"vb{t}_{k}", tag=f"vb{t}")
                    nc.scalar.copy(vbn[:, :], vn[:, :])
                    nc.scalar.copy(vn[:, :], vbn[:, :])

                    x_t[t], x3_t[t], v_t[t], vb_t[t] = xn, nx3, vn, vbn

            # ---- final half-step + output ----
            for t in range(NTILES):
                cs = slice(t * TPE, (t + 1) * TPE)
                xn = roll.tile([N, TPE], f32, name=f"xf{t}", tag=f"x{t}")
                nc.vector.tensor_sub(xn[:, :], x_t[t][:, :], v_t[t][:, :])
                nc.sync.dma_start(out_d[:, cs], xn[:, :])

    nc.compile()
    return nc


def _get_compiled():
    global _compiled
    if _compiled is None:
        _compiled = _build()
    return _compiled


def _prep_inputs(x, A, b, omega):
    """Host-side shard + layout prep. Returns list of per-core in_maps."""
    A = np.ascontiguousarray(A, dtype=np.float32)
    x = np.asarray(x, dtype=np.float32)
    b = np.asarray(b, dtype=np.float32)
    omega = np.asarray(omega, dtype=np.float32)

    A1 = A.astype(_BF16)
    A2 = (A - A1.astype(np.float32)).astype(_BF16)
    dA = np.ascontiguousarray(np.diagonal(A, axis1=1, axis2=2))

    in_maps = []
    for c in range(NCORES):
        sl = slice(c * PER_CORE, (c + 1) * PER_CORE)
        m = {}
        for t in range(NTILES):
            ts = slice(c * PER_CORE + t * TPE, c * PER_CORE + (t + 1) * TPE)
            # lhsT layout [j, (e, i)]: element e's weights = A[e].T
            m[f"at1_{t}"] = np.ascontiguousarray(
                A1[ts].transpose(2, 0, 1)
            ).reshape(N, TPE * N)
            m[f"at2_{t}"] = np.ascontiguousarray(
                A2[ts].transpose(2, 0, 1)
            ).reshape(N, TPE * N)
        m["x0t"] = np.ascontiguousarray(x[sl].T)
        m["bt"] = np.ascontiguousarray(b[sl].T)
        m["dat"] = np.ascontiguousarray(dA[sl].T)
        m["omt"] = np.ascontiguousarray(
            np.broadcast_to(omega[sl].reshape(1, PER_CORE), (N, PER_CORE))
        )
        in_maps.append(m)
    return in_maps


def _run(inputs, trace=False):
    from concourse.bass_utils import run_bass_kernel_spmd

    nc = _get_compiled()
    in_maps = _prep_inputs(
        inputs["x"], inputs["A"], inputs["b"], inputs["omega"]
    )
    res = run_bass_kernel_spmd(
        nc, in_maps, core_ids=list(range(NCORES)), trace=trace
    )
    out = np.empty((BATCH, N), dtype=np.float32)
    for c in range(NCORES):
        out[c * PER_CORE : (c + 1) * PER_CORE] = res.results[c]["outt"].T
    return out, res


def kernel(x, A, b, omega):
    out, _ = _run({"x": x, "A": A, "b": b, "omega": omega}, trace=False)
    return out


# revision 11
# speedup vs baseline: 1.4502x; 1.0804x over previous
"""Newton-SOR batched solver for Trainium2, 8 NeuronCores, data parallel.

Math: the reference's while-loop always runs all MAXITER=16 iterations
(the fp32 residual-norm floor ~5e-5 never reaches TOL=1e-6), and the
iterate converges to the fixed point F(x*)=0, so an approximate-but-
convergent inner solve reproduces the reference to ~1e-5 relative.

Per outer iteration (damped Newton-Jacobi, K=1 Neumann):
    d~ = diag(A) + 3 x^2
    v  = omega * F / d~      (rounded to bf16; the update uses the SAME
                              rounded vector, keeping F exactly consistent)
    x' = x - v
    F' = F - A @ v + (x'^3 - x^3)

The heavy op is 2048 independent 128x128 matvecs per iteration. They run
on TensorE as bf16 self-loading matmuls (N=1) with fp32 PSUM accumulation
(~32ns/element steady): A = A1 + A2, both bf16; F is carried with
A1-only applies and the *exactly linear* deferred part A2 @ (sum(v)-x0)
is folded in at a few correction iterations (drift contracts afterwards).
Everything stays in transposed layout [var, element] so TensorE needs no
transposes; VectorE/ScalarE pointwise work is hoisted off the PSUM
critical path so it hides under TensorE's stream. The 16th iteration
needs no matvec at all (F_16 is never consumed).
"""

import numpy as np
import ml_dtypes

BATCH = 2048
N = 128
NCORES = 8
PER_CORE = BATCH // NCORES          # 256
NTILES = 2                          # halves of 128 elements each
TPE = PER_CORE // NTILES            # 128 elements per tile
NITER = 16
# Elements are globally sorted by omega: tile0 gets the slow-converging
# (low omega) half and runs 15 applies; tile1 gets the fast half and
# needs only 10 (validated: total rel err ~6e-6 either way).
NAPPLY_T = (15, 10)
CORR_T = (frozenset({6, 11, 15}), frozenset({6, 10}))
RECIP_FULL = 2                      # full reciprocal for k <= this
RECIP_NEWTON = 5                    # one Newton refresh for k <= this
NCHUNK = 4                          # DMA chunks per A1 tile

_BF16 = ml_dtypes.bfloat16

_compiled = None


def _build():
    import concourse.bacc as bacc
    import concourse.mybir as mybir
    from concourse.tile import TileContext

    f32 = mybir.dt.float32
    bf16 = mybir.dt.bfloat16
    op = mybir.AluOpType

    nc = bacc.Bacc("TRN2", target_bir_lowering=False, debug=False)

    at1 = [
        nc.dram_tensor(f"at1_{t}", [N, TPE * N], bf16, kind="ExternalInput")
        for t in range(NTILES)
    ]
    at2 = [
        nc.dram_tensor(f"at2_{t}", [N, TPE * N], bf16, kind="ExternalInput")
        for t in range(NTILES)
    ]
    x0_d = nc.dram_tensor("x0t", [N, PER_CORE], f32, kind="ExternalInput")
    b_d = nc.dram_tensor("bt", [N, PER_CORE], f32, kind="ExternalInput")
    da_d = nc.dram_tensor("dat", [N, PER_CORE], f32, kind="ExternalInput")
    om_d = nc.dram_tensor("omt", [N, PER_CORE], f32, kind="ExternalInput")
    out_d = nc.dram_tensor("outt", [N, PER_CORE], f32, kind="ExternalOutput")

    with TileContext(nc) as tc:
        with (
            tc.tile_pool(name="wts", bufs=1) as wts,
            tc.tile_pool(name="vec", bufs=1) as vec,
            tc.tile_pool(name="roll", bufs=2) as roll,
            tc.tile_pool(name="ps", bufs=2, space="PSUM") as psp,
        ):
            # small vectors first so pointwise prep can start immediately
            x0_sb = vec.tile([N, PER_CORE], f32, name="x0sb")
            nc.sync.dma_start(x0_sb[:, :], x0_d[:, :])
            b_sb = vec.tile([N, PER_CORE], f32, name="bsb")
            nc.sync.dma_start(b_sb[:, :], b_d[:, :])
            da_sb = vec.tile([N, PER_CORE], f32, name="dasb")
            nc.sync.dma_start(da_sb[:, :], da_d[:, :])
            om_sb = vec.tile([N, PER_CORE], f32, name="omsb")
            nc.sync.dma_start(om_sb[:, :], om_d[:, :])

            # A1 weights, chunked, alternating between both DMA engines so
            # tile0 completes first and PE starts as early as possible.
            a1_sb = []
            for t in range(NTILES):
                a1_t = wts.tile([N, TPE * N], bf16, name=f"a1sb{t}", tag=f"a1{t}")
                a1_sb.append(a1_t)
            csz = TPE * N // NCHUNK
            engs = (nc.sync, nc.gpsimd)
            qi = 0
            for t in range(NTILES):
                for q in range(NCHUNK):
                    engs[qi % 2].dma_start(
                        a1_sb[t][:, q * csz : (q + 1) * csz],
                        at1[t][:, q * csz : (q + 1) * csz],
                    )
                    qi += 1
            a2_sb = []
            for t in range(NTILES):
                a2_t = wts.tile([N, TPE * N], bf16, name=f"a2sb{t}", tag=f"a2{t}")
                engs[t % 2].dma_start(a2_t[:, :], at2[t][:, :])
                a2_sb.append(a2_t)

            def apply_mms(ps, a_sb, v_bf):
                for e in range(TPE):
                    nc.tensor.matmul(
                        ps[:, e : e + 1],
                        a_sb[:, e * N : (e + 1) * N],
                        v_bf[:, e : e + 1],
                        start=True,
                        stop=True,
                    )

            # per-tile persistent state
            F_t = [vec.tile([N, TPE], f32, name=f"F{t}") for t in range(2)]
            wa_t = [vec.tile([N, TPE], f32, name=f"wa{t}") for t in range(2)]
            r_t = [vec.tile([N, TPE], f32, name=f"r{t}") for t in range(2)]
            s_t = [vec.tile([N, TPE], f32, name=f"s{t}") for t in range(2)]
            x_t = [None] * NTILES
            x3_t = [None] * NTILES
            v_t = [None] * NTILES
            vb_t = [None] * NTILES

            # ---- init per tile ----
            for t in range(NTILES):
                cs = slice(t * TPE, (t + 1) * TPE)
                xb = roll.tile([N, TPE], bf16, name=f"xb{t}", tag=f"vb{t}")
                nc.scalar.copy(xb[:, :], x0_sb[:, cs])
                x = roll.tile([N, TPE], f32, name=f"x{t}", tag=f"x{t}")
                nc.scalar.copy(x[:, :], xb[:, :])          # x = round(x0)
                ps = psp.tile([N, TPE], f32, name=f"psi{t}", tag=f"ps{t}")
                apply_mms(ps, a1_sb[t], xb)
                # wacc = -x
                nc.vector.tensor_scalar_mul(wa_t[t][:, :], x[:, :], -1.0)
                x2 = roll.tile([N, TPE], f32, name=f"x2{t}", tag=f"x2{t}")
                nc.scalar.square(x2[:, :], x[:, :])
                x3 = roll.tile([N, TPE], f32, name=f"x3{t}", tag=f"x3{t}")
                nc.vector.tensor_mul(x3[:, :], x2[:, :], x[:, :])
                # d~, r, s
                dt_ = roll.tile([N, TPE], f32, name=f"dt{t}", tag=f"dt{t}")
                nc.vector.scalar_tensor_tensor(
                    dt_[:, :], x2[:, :], 3.0, da_sb[:, cs],
                    op0=op.mult, op1=op.add,
                )
                nc.vector.reciprocal(r_t[t][:, :], dt_[:, :])
                nc.vector.tensor_mul(s_t[t][:, :], r_t[t][:, :], om_sb[:, cs])
                # F = (x3 - b) + ps
                nc.vector.tensor_sub(F_t[t][:, :], x3[:, :], b_sb[:, cs])
                nc.vector.tensor_add(F_t[t][:, :], F_t[t][:, :], ps[:, :])
                # v_1
                v = roll.tile([N, TPE], f32, name=f"v{t}", tag=f"v{t}")
                nc.vector.tensor_mul(v[:, :], F_t[t][:, :], s_t[t][:, :])
                v_bf = roll.tile([N, TPE], bf16, name=f"vb{t}", tag=f"vb{t}")
                nc.scalar.copy(v_bf[:, :], v[:, :])
                nc.scalar.copy(v[:, :], v_bf[:, :])        # rounded, f32
                x_t[t], x3_t[t], v_t[t], vb_t[t] = x, x3, v, v_bf

            # ---- iterations (last one per tile needs no apply) ----
            for k in range(1, max(NAPPLY_T) + 1):
                for t in range(NTILES):
                    if k > NAPPLY_T[t]:
                        continue
                    corr = k in CORR_T[t]
                    cs = slice(t * TPE, (t + 1) * TPE)
                    x, x3, v, v_bf = x_t[t], x3_t[t], v_t[t], vb_t[t]
                    F, wa, r, s = F_t[t], wa_t[t], r_t[t], s_t[t]

                    ps = psp.tile([N, TPE], f32, name=f"psk{t}_{k}", tag=f"ps{t}")
                    apply_mms(ps, a1_sb[t], v_bf)

                    # --- hoisted pointwise (runs under the PE stream) ---
                    xn = roll.tile([N, TPE], f32, name=f"x{t}_{k}", tag=f"x{t}")
                    nc.vector.tensor_sub(xn[:, :], x[:, :], v[:, :])
                    x2 = roll.tile([N, TPE], f32, name=f"x2{t}_{k}", tag=f"x2{t}")
                    nc.scalar.square(x2[:, :], xn[:, :])
                    nx3 = roll.tile([N, TPE], f32, name=f"x3{t}_{k}", tag=f"x3{t}")
                    nc.vector.tensor_mul(nx3[:, :], x2[:, :], xn[:, :])
                    dc = roll.tile([N, TPE], f32, name=f"dc{t}_{k}", tag=f"dt{t}")
                    nc.vector.tensor_sub(dc[:, :], nx3[:, :], x3[:, :])
                    nc.vector.tensor_add(F[:, :], F[:, :], dc[:, :])
                    ps2 = None
                    if corr:
                        w32 = roll.tile([N, TPE], f32, name=f"w{t}_{k}", tag=f"w{t}")
                        nc.vector.tensor_add(w32[:, :], wa[:, :], v[:, :])
                        w_bf = roll.tile(
                            [N, TPE], bf16, name=f"wb{t}_{k}", tag=f"wb{t}"
                        )
                        nc.scalar.copy(w_bf[:, :], w32[:, :])
                        wr = roll.tile([N, TPE], f32, name=f"wr{t}_{k}", tag=f"w{t}")
                        nc.scalar.copy(wr[:, :], w_bf[:, :])
                        nc.vector.tensor_sub(wa[:, :], w32[:, :], wr[:, :])
                        ps2 = psp.tile(
                            [N, TPE], f32, name=f"psc{t}_{k}", tag=f"pc{t}"
                        )
                        apply_mms(ps2, a2_sb[t], w_bf)
                    else:
                        nc.gpsimd.tensor_add(wa[:, :], wa[:, :], v[:, :])
                    # d~(x'), reciprocal policy, s
                    if k + 1 <= RECIP_NEWTON:
                        dt_ = roll.tile(
                            [N, TPE], f32, name=f"dt{t}_{k}", tag=f"dt{t}"
                        )
                        nc.vector.scalar_tensor_tensor(
                            dt_[:, :], x2[:, :], 3.0, da_sb[:, cs],
                            op0=op.mult, op1=op.add,
                        )
                        if k + 1 <= RECIP_FULL:
                            nc.vector.reciprocal(r[:, :], dt_[:, :])
                        else:
                            # r <- r*(2 - d*r)
                            tmp = roll.tile(
                                [N, TPE], f32, name=f"tm{t}_{k}", tag=f"tm{t}"
                            )
                            nc.vector.tensor_mul(tmp[:, :], dt_[:, :], r[:, :])
                            nc.vector.tensor_scalar(
                                tmp[:, :], tmp[:, :], -1.0, 2.0,
                                op0=op.mult, op1=op.add,
                            )
                            nc.vector.tensor_mul(r[:, :], r[:, :], tmp[:, :])
                        nc.vector.tensor_mul(s[:, :], r[:, :], om_sb[:, cs])

                    # --- PSUM critical path ---
                    nc.vector.tensor_sub(F[:, :], F[:, :], ps[:, :])
                    if ps2 is not None:
                        nc.vector.tensor_sub(F[:, :], F[:, :], ps2[:, :])
                    vn = roll.tile([N, TPE], f32, name=f"v{t}_{k}", tag=f"v{t}")
                    nc.vector.tensor_mul(vn[:, :], F[:, :], s[:, :])
                    vbn = roll.tile([N, TPE], bf16, name=f"vb{t}_{k}", tag=f"vb{t}")
                    nc.vector.tensor_copy(vbn[:, :], vn[:, :])
                    nc.scalar.copy(vn[:, :], vbn[:, :])

                    x_t[t], x3_t[t], v_t[t], vb_t[t] = xn, nx3, vn, vbn

            # ---- final half-step + output ----
            for t in range(NTILES):
                cs = slice(t * TPE, (t + 1) * TPE)
                xn = roll.tile([N, TPE], f32, name=f"xf{t}", tag=f"x{t}")
                nc.vector.tensor_sub(xn[:, :], x_t[t][:, :], v_t[t][:, :])
                nc.sync.dma_start(out_d[:, cs], xn[:, :])

    nc.compile()
    return nc


def _get_compiled():
    global _compiled
    if _compiled is None:
        _compiled = _build()
    return _compiled


def _perm_for(omega):
    """Global omega sort: slow (low omega) half feeds every core's tile0,
    fast half feeds tile1. perm[slot] = source batch index."""
    order = np.argsort(np.asarray(omega, dtype=np.float32)[:, 0], kind="stable")
    half = BATCH // 2
    perm = np.empty(BATCH, dtype=np.int64)
    for c in range(NCORES):
        perm[c * PER_CORE : c * PER_CORE + TPE] = order[c * TPE : (c + 1) * TPE]
        perm[c * PER_CORE + TPE : (c + 1) * PER_CORE] = order[
            half + c * TPE : half + (c + 1) * TPE
        ]
    return perm


def _prep_inputs(x, A, b, omega, perm):
    """Host-side shard + layout prep. Returns list of per-core in_maps."""
    A = np.ascontiguousarray(A, dtype=np.float32)
    x = np.asarray(x, dtype=np.float32)[perm]
    b = np.asarray(b, dtype=np.float32)[perm]
    omega = np.asarray(omega, dtype=np.float32)[perm]

    Ap = A[perm]
    A1 = Ap.astype(_BF16)
    A2 = (Ap - A1.astype(np.float32)).astype(_BF16)
    dA = np.ascontiguousarray(np.diagonal(Ap, axis1=1, axis2=2))

    in_maps = []
    for c in range(NCORES):
        sl = slice(c * PER_CORE, (c + 1) * PER_CORE)
        m = {}
        for t in range(NTILES):
            ts = slice(c * PER_CORE + t * TPE, c * PER_CORE + (t + 1) * TPE)
            # lhsT layout [j, (e, i)]: element e's weights = A[e].T
            m[f"at1_{t}"] = np.ascontiguousarray(
                A1[ts].transpose(2, 0, 1)
            ).reshape(N, TPE * N)
            m[f"at2_{t}"] = np.ascontiguousarray(
                A2[ts].transpose(2, 0, 1)
            ).reshape(N, TPE * N)
        m["x0t"] = np.ascontiguousarray(x[sl].T)
        m["bt"] = np.ascontiguousarray(b[sl].T)
        m["dat"] = np.ascontiguousarray(dA[sl].T)
        m["omt"] = np.ascontiguousarray(
            np.broadcast_to(omega[sl].reshape(1, PER_CORE), (N, PER_CORE))
        )
        in_maps.append(m)
    return in_maps


def _run(inputs, trace=False):
    from concourse.bass_utils import run_bass_kernel_spmd

    nc = _get_compiled()
    perm = _perm_for(inputs["omega"])
    in_maps = _prep_inputs(
        inputs["x"], inputs["A"], inputs["b"], inputs["omega"], perm
    )
    res = run_bass_kernel_spmd(
        nc, in_maps, core_ids=list(range(NCORES)), trace=trace
    )
    out = np.empty((BATCH, N), dtype=np.float32)
    for c in range(NCORES):
        out[perm[c * PER_CORE : (c + 1) * PER_CORE]] = res.results[c]["outt"].T
    return out, res


def kernel(x, A, b, omega):
    out, _ = _run({"x": x, "A": A, "b": b, "omega": omega}, trace=False)
    return out


# revision 17
# speedup vs baseline: 1.6442x; 1.1337x over previous
"""Newton-SOR batched solver for Trainium2, 8 NeuronCores, data parallel.

Math: the reference's while-loop always runs all MAXITER=16 iterations
(the fp32 residual-norm floor ~5e-5 never reaches TOL=1e-6), and the
iterate converges to the fixed point F(x*)=0, so an approximate-but-
convergent inner solve reproduces the reference to ~1e-5 relative.

Per outer iteration (damped Newton-Jacobi, K=1 Neumann):
    d~ = diag(A) + 3 x^2
    v  = omega * F / d~      (rounded to bf16; the update uses the SAME
                              rounded vector, keeping F exactly consistent)
    x' = x - v
    F' = F - A @ v + (x'^3 - x^3)

The heavy op is 2048 independent 128x128 matvecs per iteration. They run
on TensorE as bf16 self-loading matmuls (N=1) with fp32 PSUM accumulation
(~32ns/element steady): A = A1 + A2, both bf16; F is carried with
A1-only applies and the *exactly linear* deferred part A2 @ (sum(v)-x0)
is folded in at a few correction iterations (drift contracts afterwards).
Everything stays in transposed layout [var, element] so TensorE needs no
transposes; VectorE/ScalarE pointwise work is hoisted off the PSUM
critical path so it hides under TensorE's stream. The 16th iteration
needs no matvec at all (F_16 is never consumed).
"""

import numpy as np
import ml_dtypes

BATCH = 2048
N = 128
NCORES = 8
PER_CORE = BATCH // NCORES          # 256
NTILES = 2                          # halves of 128 elements each
TPE = PER_CORE // NTILES            # 128 elements per tile
NITER = 16
# Elements are globally sorted by omega: tile0 gets the slow-converging
# (low omega) half and runs 15 applies; tile1 gets the fast half and
# needs only 10 (validated: total rel err ~6e-6 either way).
NAPPLY_T = (15, 10)
CORR_T = (frozenset({6, 11, 15}), frozenset({6, 10}))
RECIP_FULL = 2                      # full reciprocal for k <= this
RECIP_NEWTON = 5                    # one Newton refresh for k <= this
NCHUNK = 4                          # DMA chunks per A1 tile
NHALF = 2                           # column-halves for PSUM critical path
HTPE = TPE // NHALF

_BF16 = ml_dtypes.bfloat16

_compiled = None


def _build():
    import concourse.bacc as bacc
    import concourse.mybir as mybir
    from concourse.tile import TileContext

    f32 = mybir.dt.float32
    bf16 = mybir.dt.bfloat16
    op = mybir.AluOpType

    nc = bacc.Bacc("TRN2", target_bir_lowering=False, debug=False)

    at1 = [
        nc.dram_tensor(f"at1_{t}", [N, TPE * N], bf16, kind="ExternalInput")
        for t in range(NTILES)
    ]
    at2 = [
        nc.dram_tensor(f"at2_{t}", [N, TPE * N], bf16, kind="ExternalInput")
        for t in range(NTILES)
    ]
    x0_d = nc.dram_tensor("x0t", [N, PER_CORE], f32, kind="ExternalInput")
    b_d = nc.dram_tensor("bt", [N, PER_CORE], f32, kind="ExternalInput")
    da_d = nc.dram_tensor("dat", [N, PER_CORE], f32, kind="ExternalInput")
    om_d = nc.dram_tensor("omt", [N, PER_CORE], f32, kind="ExternalInput")
    out_d = nc.dram_tensor("outt", [N, PER_CORE], f32, kind="ExternalOutput")

    with TileContext(nc) as tc:
        with (
            tc.tile_pool(name="wts", bufs=1) as wts,
            tc.tile_pool(name="vec", bufs=1) as vec,
            tc.tile_pool(name="roll", bufs=2) as roll,
            tc.tile_pool(name="ps", bufs=2, space="PSUM") as psp,
        ):
            # small vectors first so pointwise prep can start immediately
            x0_sb = vec.tile([N, PER_CORE], f32, name="x0sb")
            nc.sync.dma_start(x0_sb[:, :], x0_d[:, :])
            b_sb = vec.tile([N, PER_CORE], f32, name="bsb")
            nc.sync.dma_start(b_sb[:, :], b_d[:, :])
            da_sb = vec.tile([N, PER_CORE], f32, name="dasb")
            nc.sync.dma_start(da_sb[:, :], da_d[:, :])
            om_sb = vec.tile([N, PER_CORE], f32, name="omsb")
            nc.sync.dma_start(om_sb[:, :], om_d[:, :])

            # Bulk weights go on the gpsimd SWDGE queue (~250GB/s measured;
            # the sync HWDGE queue trickles at ~50GB/s, so it only carries
            # the small vectors above and the last-needed A2 tile).
            a1_sb = []
            for t in range(NTILES):
                a1_t = wts.tile([N, TPE * N], bf16, name=f"a1sb{t}", tag=f"a1{t}")
                a1_sb.append(a1_t)
            csz = TPE * N // NCHUNK
            for t in range(NTILES):
                for q in range(NCHUNK):
                    nc.gpsimd.dma_start(
                        a1_sb[t][:, q * csz : (q + 1) * csz],
                        at1[t][:, q * csz : (q + 1) * csz],
                    )
            a2_sb = []
            for t in range(NTILES):
                a2_t = wts.tile([N, TPE * N], bf16, name=f"a2sb{t}", tag=f"a2{t}")
                nc.gpsimd.dma_start(a2_t[:, :], at2[t][:, :])
                a2_sb.append(a2_t)

            def apply_mms(ps, a_sb, v_bf, e0=0, e1=TPE):
                for e in range(e0, e1):
                    nc.tensor.matmul(
                        ps[:, e : e + 1],
                        a_sb[:, e * N : (e + 1) * N],
                        v_bf[:, e : e + 1],
                        start=True,
                        stop=True,
                    )

            # per-tile persistent state
            F_t = [vec.tile([N, TPE], f32, name=f"F{t}") for t in range(2)]
            wa_t = [vec.tile([N, TPE], f32, name=f"wa{t}") for t in range(2)]
            r_t = [vec.tile([N, TPE], f32, name=f"r{t}") for t in range(2)]
            s_t = [vec.tile([N, TPE], f32, name=f"s{t}") for t in range(2)]
            x_t = [None] * NTILES
            x3_t = [None] * NTILES
            v_t = [None] * NTILES
            vb_t = [None] * NTILES

            # ---- init per tile ----
            for t in range(NTILES):
                cs = slice(t * TPE, (t + 1) * TPE)
                xb = roll.tile([N, TPE], bf16, name=f"xb{t}", tag=f"vb{t}")
                nc.scalar.copy(xb[:, :], x0_sb[:, cs])
                x = roll.tile([N, TPE], f32, name=f"x{t}", tag=f"x{t}")
                nc.scalar.copy(x[:, :], xb[:, :])          # x = round(x0)
                ps = psp.tile([N, TPE], f32, name=f"psi{t}", tag=f"ps{t}")
                apply_mms(ps, a1_sb[t], xb)
                # wacc = -x
                nc.vector.tensor_scalar_mul(wa_t[t][:, :], x[:, :], -1.0)
                x2 = roll.tile([N, TPE], f32, name=f"x2{t}", tag=f"x2{t}")
                nc.scalar.square(x2[:, :], x[:, :])
                x3 = roll.tile([N, TPE], f32, name=f"x3{t}", tag=f"x3{t}")
                nc.vector.tensor_mul(x3[:, :], x2[:, :], x[:, :])
                # d~, r, s
                dt_ = roll.tile([N, TPE], f32, name=f"dt{t}", tag=f"dt{t}")
                nc.vector.scalar_tensor_tensor(
                    dt_[:, :], x2[:, :], 3.0, da_sb[:, cs],
                    op0=op.mult, op1=op.add,
                )
                nc.vector.reciprocal(r_t[t][:, :], dt_[:, :])
                nc.vector.tensor_mul(s_t[t][:, :], r_t[t][:, :], om_sb[:, cs])
                # F = (x3 - b) + ps
                nc.vector.tensor_sub(F_t[t][:, :], x3[:, :], b_sb[:, cs])
                nc.vector.tensor_add(F_t[t][:, :], F_t[t][:, :], ps[:, :])
                # v_1 (bf16 directly; all consumers upconvert exactly)
                v_bf = roll.tile([N, TPE], bf16, name=f"vb{t}", tag=f"vb{t}")
                nc.vector.tensor_mul(v_bf[:, :], F_t[t][:, :], s_t[t][:, :])
                x_t[t], x3_t[t], vb_t[t] = x, x3, v_bf

            # ---- iterations (last one per tile needs no apply) ----
            for k in range(1, max(NAPPLY_T) + 1):
                for t in range(NTILES):
                    if k > NAPPLY_T[t]:
                        continue
                    corr = k in CORR_T[t]
                    cs = slice(t * TPE, (t + 1) * TPE)
                    x, x3, v_bf = x_t[t], x3_t[t], vb_t[t]
                    F, wa, r, s = F_t[t], wa_t[t], r_t[t], s_t[t]

                    ps = psp.tile([N, TPE], f32, name=f"psk{t}_{k}", tag=f"ps{t}")
                    ps2 = None
                    w32 = w_bf = None
                    if corr:
                        # w-chain first so the A2 matmuls aren't starved
                        w32 = roll.tile([N, TPE], f32, name=f"w{t}_{k}", tag=f"w{t}")
                        nc.vector.tensor_add(w32[:, :], wa[:, :], v_bf[:, :])
                        w_bf = roll.tile(
                            [N, TPE], bf16, name=f"wb{t}_{k}", tag=f"wb{t}"
                        )
                        nc.scalar.copy(w_bf[:, :], w32[:, :])
                        ps2 = psp.tile(
                            [N, TPE], f32, name=f"psc{t}_{k}", tag=f"pc{t}"
                        )
                    apply_mms(ps, a1_sb[t], v_bf)
                    if corr:
                        apply_mms(ps2, a2_sb[t], w_bf)

                    # --- hoisted pointwise (runs under the PE stream) ---
                    xn = roll.tile([N, TPE], f32, name=f"x{t}_{k}", tag=f"x{t}")
                    nc.vector.tensor_sub(xn[:, :], x[:, :], v_bf[:, :])
                    x2 = roll.tile([N, TPE], f32, name=f"x2{t}_{k}", tag=f"x2{t}")
                    nc.scalar.square(x2[:, :], xn[:, :])
                    nx3 = roll.tile([N, TPE], f32, name=f"x3{t}_{k}", tag=f"x3{t}")
                    nc.vector.tensor_mul(nx3[:, :], x2[:, :], xn[:, :])
                    dc = roll.tile([N, TPE], f32, name=f"dc{t}_{k}", tag=f"dt{t}")
                    nc.vector.tensor_sub(dc[:, :], nx3[:, :], x3[:, :])
                    nc.vector.tensor_add(F[:, :], F[:, :], dc[:, :])
                    if corr:
                        nc.vector.tensor_sub(wa[:, :], w32[:, :], w_bf[:, :])
                    else:
                        nc.gpsimd.tensor_add(wa[:, :], wa[:, :], v_bf[:, :])
                    # d~(x'), reciprocal policy, s
                    if k + 1 <= RECIP_NEWTON:
                        dt_ = roll.tile(
                            [N, TPE], f32, name=f"dt{t}_{k}", tag=f"dt{t}"
                        )
                        nc.vector.scalar_tensor_tensor(
                            dt_[:, :], x2[:, :], 3.0, da_sb[:, cs],
                            op0=op.mult, op1=op.add,
                        )
                        if k + 1 <= RECIP_FULL:
                            nc.vector.reciprocal(r[:, :], dt_[:, :])
                        else:
                            # r <- r*(2 - d*r)
                            tmp = roll.tile(
                                [N, TPE], f32, name=f"tm{t}_{k}", tag=f"tm{t}"
                            )
                            nc.vector.tensor_mul(tmp[:, :], dt_[:, :], r[:, :])
                            nc.vector.tensor_scalar(
                                tmp[:, :], tmp[:, :], -1.0, 2.0,
                                op0=op.mult, op1=op.add,
                            )
                            nc.vector.tensor_mul(r[:, :], r[:, :], tmp[:, :])
                        nc.vector.tensor_mul(s[:, :], r[:, :], om_sb[:, cs])

                    # --- PSUM critical path, pipelined per column-half ---
                    vbn = roll.tile([N, TPE], bf16, name=f"vb{t}_{k}", tag=f"vb{t}")
                    for h in range(NHALF):
                        hs = slice(h * HTPE, (h + 1) * HTPE)
                        nc.vector.tensor_sub(F[:, hs], F[:, hs], ps[:, hs])
                        if ps2 is not None:
                            nc.vector.tensor_sub(F[:, hs], F[:, hs], ps2[:, hs])
                        nc.vector.tensor_mul(vbn[:, hs], F[:, hs], s[:, hs])

                    x_t[t], x3_t[t], vb_t[t] = xn, nx3, vbn

            # ---- final half-step + output ----
            for t in range(NTILES):
                cs = slice(t * TPE, (t + 1) * TPE)
                xn = roll.tile([N, TPE], f32, name=f"xf{t}", tag=f"x{t}")
                nc.vector.tensor_sub(xn[:, :], x_t[t][:, :], vb_t[t][:, :])
                nc.sync.dma_start(out_d[:, cs], xn[:, :])

    nc.compile()
    return nc


def _get_compiled():
    global _compiled
    if _compiled is None:
        _compiled = _build()
    return _compiled


def _perm_for(omega):
    """Global omega sort: slow (low omega) half feeds every core's tile0,
    fast half feeds tile1. perm[slot] = source batch index."""
    order = np.argsort(np.asarray(omega, dtype=np.float32)[:, 0], kind="stable")
    half = BATCH // 2
    perm = np.empty(BATCH, dtype=np.int64)
    for c in range(NCORES):
        perm[c * PER_CORE : c * PER_CORE + TPE] = order[c * TPE : (c + 1) * TPE]
        perm[c * PER_CORE + TPE : (c + 1) * PER_CORE] = order[
            half + c * TPE : half + (c + 1) * TPE
        ]
    return perm


def _prep_inputs(x, A, b, omega, perm):
    """Host-side shard + layout prep. Returns list of per-core in_maps."""
    A = np.ascontiguousarray(A, dtype=np.float32)
    x = np.asarray(x, dtype=np.float32)[perm]
    b = np.asarray(b, dtype=np.float32)[perm]
    omega = np.asarray(omega, dtype=np.float32)[perm]

    Ap = A[perm]
    A1 = Ap.astype(_BF16)
    A2 = (Ap - A1.astype(np.float32)).astype(_BF16)
    dA = np.ascontiguousarray(np.diagonal(Ap, axis1=1, axis2=2))

    in_maps = []
    for c in range(NCORES):
        sl = slice(c * PER_CORE, (c + 1) * PER_CORE)
        m = {}
        for t in range(NTILES):
            ts = slice(c * PER_CORE + t * TPE, c * PER_CORE + (t + 1) * TPE)
            # lhsT layout [j, (e, i)]: element e's weights = A[e].T
            m[f"at1_{t}"] = np.ascontiguousarray(
                A1[ts].transpose(2, 0, 1)
            ).reshape(N, TPE * N)
            m[f"at2_{t}"] = np.ascontiguousarray(
                A2[ts].transpose(2, 0, 1)
            ).reshape(N, TPE * N)
        m["x0t"] = np.ascontiguousarray(x[sl].T)
        m["bt"] = np.ascontiguousarray(b[sl].T)
        m["dat"] = np.ascontiguousarray(dA[sl].T)
        m["omt"] = np.ascontiguousarray(
            np.broadcast_to(omega[sl].reshape(1, PER_CORE), (N, PER_CORE))
        )
        in_maps.append(m)
    return in_maps


def _run(inputs, trace=False):
    from concourse.bass_utils import run_bass_kernel_spmd

    nc = _get_compiled()
    perm = _perm_for(inputs["omega"])
    in_maps = _prep_inputs(
        inputs["x"], inputs["A"], inputs["b"], inputs["omega"], perm
    )
    res = run_bass_kernel_spmd(
        nc, in_maps, core_ids=list(range(NCORES)), trace=trace
    )
    out = np.empty((BATCH, N), dtype=np.float32)
    for c in range(NCORES):
        out[perm[c * PER_CORE : (c + 1) * PER_CORE]] = res.results[c]["outt"].T
    return out, res


def kernel(x, A, b, omega):
    out, _ = _run({"x": x, "A": A, "b": b, "omega": omega}, trace=False)
    return out
